# revision 9
# baseline (speedup 1.0000x reference)
"""Multi-head attention (torch-bug-faithful) Bass/Tile kernel for 8 trn2 cores.

Reference math (after the torch .reshape reinterpretation):
  X2d = query.reshape(4096, 1024)                    # rows r = l*4 + n
  Y   = X2d @ W_in.T + b_in                          # (4096, 3072)
  For "batch" b in 0..4: Yb = Y[b*1024:(b+1)*1024]   # (1024, 3072)
    head h in 0..16: q = Yb[:, h*192:h*192+64] * hd**-0.5
                     k = Yb[:, h*192+64:h*192+128]
                     v = Yb[:, h*192+128:h*192+192]
    S = q @ k.T; A = softmax(S, -1); ctx_h = A @ v
  C_b[:, h*64:(h+1)*64] = ctx_h; O_b = C_b @ W_out.T + b_out
  out1[l', n', :] = O_{n'}[l']                       # (1024, 4, 1024)
  out2[b] = sum_h A_bh / 16                          # (4, 1024, 1024)

Sharding: core c -> (b = c//2, half = c%2) handles 8 heads of one batch.
Each core computes partial O.T (features x tokens) and partial out2; the
host sums the two half-partials per batch and adds b_out.

On-chip layout: all activations transposed (features on partitions).
  in_proj: Y.T tiles = W.T-chunk.T @ X.T-chunk (PE), bias via ones-row matmul
  QK:      S (q-tile x s) = (Q.T chunk).T @ K.T      (contract d=64)
  exp:     ACT Exp PSUM->SBUF bf16 with accum_out row sums
  out2:    acc += E * recip16 (scalar_tensor_tensor, DVE + GPSIMD split)
  E.T:     DMA xbar transpose (bf16) for the A@V contraction
  AV:      ctx(q-tile, d) = (E.T chunk).T @ V-chunk  (PSUM accumulated)
  norm:    ACT Copy with per-partition scale = 1/rowsum
  out_proj: O.T = (W_out.T chunk).T @ C.T            (C.T via DMA transpose)
"""

import numpy as np
import ml_dtypes
from contextlib import ExitStack

import concourse.bass as bass
import concourse.tile as tile
import concourse.mybir as mybir
from concourse import bacc
from concourse.bass_utils import run_bass_kernel_spmd

BF16 = mybir.dt.bfloat16
F32 = mybir.dt.float32
NPBF16 = ml_dtypes.bfloat16

L, N, E = 1024, 4, 1024
HEADS_TOTAL, HD = 16, 64
T = 1024          # tokens per core
HL = 8            # heads per core
F = HL * 3 * HD   # 1536 in_proj output features per core
KC = E // 128     # 8 contraction chunks
NM = T // 128     # 8 q tiles
SCALE = float(HD) ** -0.5

_CACHE = {}


def _build_kernel(ctx: ExitStack, tc, xt, wt, wot, bvec, o_out, a_out):
    nc = tc.nc
    mult, add = mybir.AluOpType.mult, mybir.AluOpType.add
    Exp, Copy = mybir.ActivationFunctionType.Exp, mybir.ActivationFunctionType.Copy

    consts = ctx.enter_context(tc.tile_pool(name="consts", bufs=1))
    wacc = ctx.enter_context(tc.tile_pool(name="wacc", bufs=1))
    xet = ctx.enter_context(tc.tile_pool(name="xet", bufs=3))
    ytp = ctx.enter_context(tc.tile_pool(name="ytp", bufs=1))
    ep = ctx.enter_context(tc.tile_pool(name="ep", bufs=2))
    vp = ctx.enter_context(tc.tile_pool(name="vp", bufs=2))
    cp = ctx.enter_context(tc.tile_pool(name="cp", bufs=1))
    ctp = ctx.enter_context(tc.tile_pool(name="ctp", bufs=1))
    osp = ctx.enter_context(tc.tile_pool(name="osp", bufs=2))
    rsp = ctx.enter_context(tc.tile_pool(name="rsp", bufs=4))
    mmp = ctx.enter_context(tc.tile_pool(name="mmp", bufs=2, space="PSUM"))
    ctxp = ctx.enter_context(tc.tile_pool(name="ctxp", bufs=3, space="PSUM"))

    # ---- input loads -----------------------------------------------------
    wt_sb = wacc.tile([128, KC, F], BF16, tag="wacc")
    for c in range(KC):
        nc.sync.dma_start(out=wt_sb[:, c, :], in_=wt[c * 128 : (c + 1) * 128, :])
    xt_sb = xet.tile([128, KC, T], BF16, tag="xet")
    for c in range(KC):
        nc.sync.dma_start(out=xt_sb[:, c, :], in_=xt[c * 128 : (c + 1) * 128, :])
    wot_sb = consts.tile([128, 4, E], BF16)
    for c in range(4):
        nc.sync.dma_start(out=wot_sb[:, c, :], in_=wot[c * 128 : (c + 1) * 128, :])
    bias_sb = consts.tile([1, F], BF16)
    nc.sync.dma_start(out=bias_sb[:], in_=bvec[:])
    ones_sb = consts.tile([1, 512], BF16)
    nc.vector.memset(ones_sb[:], 1.0)

    # ---- in_proj: Y.T tiles (features on partitions) ---------------------
    yt_sb = ytp.tile([128, F // 128, T], BF16)
    for j in range(F // 128):
        ps = mmp.tile([128, T], F32, tag="mm")
        for nh in range(2):
            half = ps[:, nh * 512 : (nh + 1) * 512]
            for c in range(KC):
                nc.tensor.matmul(
                    half,
                    lhsT=wt_sb[:, c, j * 128 : (j + 1) * 128],
                    rhs=xt_sb[:, c, nh * 512 : (nh + 1) * 512],
                    start=(c == 0), stop=False,
                )
            nc.tensor.matmul(
                half,
                lhsT=bias_sb[0:1, j * 128 : (j + 1) * 128],
                rhs=ones_sb[0:1, :],
                start=False, stop=True,
            )
        nc.scalar.copy(out=yt_sb[:, j, :], in_=ps[:])

    # Host orders Y.T features as [all-heads Q | all-heads K | all-heads V]
    # (512-row blocks) so q/k/v of head h share base partition 64*(h%2) --
    # the PE requires lhsT and rhs to start on the same partition.
    def seg(row):  # 64-row feature segment -> (partition0, ftile)
        return row % 128, row // 128

    # ---- attention heads -------------------------------------------------
    acc = wacc.tile([128, NM, T], F32, tag="wacc")
    for h in range(HL):
        pq, fq = seg(h * 64)
        pk, fk = seg(512 + h * 64)
        pv, fv = seg(1024 + h * 64)

        e_h = ep.tile([128, NM, T], BF16)
        rs_h = rsp.tile([128, NM], F32, tag="rs")
        for m in range(NM):
            ps = mmp.tile([128, T], F32, tag="mm")
            for nh in range(2):
                nc.tensor.matmul(
                    ps[:, nh * 512 : (nh + 1) * 512],
                    lhsT=yt_sb[pq : pq + 64, fq, m * 128 : (m + 1) * 128],
                    rhs=yt_sb[pk : pk + 64, fk, nh * 512 : (nh + 1) * 512],
                    start=True, stop=True,
                )
            nc.scalar.activation(
                out=e_h[:, m, :], in_=ps[:], func=Exp,
                accum_out=rs_h[:, m : m + 1],
            )

        recip_h = rsp.tile([128, NM], F32, tag="recip")
        recip16_h = rsp.tile([128, NM], F32, tag="recip16")
        nc.vector.reciprocal(out=recip_h[:], in_=rs_h[:])
        nc.vector.tensor_scalar_mul(recip16_h[:], recip_h[:], 1.0 / 16.0)

        # out2 partial: acc[:, m, :] (+)= E * (1/(16*rowsum))
        for m in range(NM):
            eng = nc.vector
            if h == 0:
                eng.tensor_scalar_mul(
                    acc[:, m, :], e_h[:, m, :], recip16_h[:, m : m + 1]
                )
            else:
                eng.scalar_tensor_tensor(
                    out=acc[:, m, :], in0=e_h[:, m, :],
                    scalar=recip16_h[:, m : m + 1], in1=acc[:, m, :],
                    op0=mult, op1=add,
                )

        # V (s on partitions) and E.T via xbar transpose
        v_h = vp.tile([128, NM, HD], BF16)
        nc.sync.dma_start_transpose(out=v_h[:], in_=yt_sb[pv : pv + 64, fv, :])
        et_h = xet.tile([128, NM, T], BF16, tag="xet")
        for m in range(NM):
            nc.sync.dma_start_transpose(
                out=et_h[:, :, m * 128 : (m + 1) * 128], in_=e_h[:, m, :]
            )

        # AV: ctx(q-tile, d), PSUM-accumulated over s chunks
        cx = ctxp.tile([128, NM, HD], F32)
        for m in range(NM):
            for t in range(NM):
                nc.tensor.matmul(
                    cx[:, m, :],
                    lhsT=et_h[:, t, m * 128 : (m + 1) * 128],
                    rhs=v_h[:, t, :],
                    start=(t == 0), stop=(t == NM - 1),
                )
        # normalize rows and pack C (q on partitions, 512 ctx features)
        if h == 0:
            c_sb = cp.tile([128, NM, 512], BF16)
        for m in range(NM):
            nc.scalar.activation(
                out=c_sb[:, m, h * HD : (h + 1) * HD], in_=cx[:, m, :],
                func=Copy, scale=recip_h[:, m : m + 1],
            )

    # ---- out_proj --------------------------------------------------------
    ct_sb = ctp.tile([128, 4, T], BF16)
    for m in range(NM):
        nc.sync.dma_start_transpose(
            out=ct_sb[:, :, m * 128 : (m + 1) * 128], in_=c_sb[:, m, :]
        )
    for j in range(E // 128):
        ps = mmp.tile([128, T], F32, tag="mm")
        for nh in range(2):
            for t in range(4):
                nc.tensor.matmul(
                    ps[:, nh * 512 : (nh + 1) * 512],
                    lhsT=wot_sb[:, t, j * 128 : (j + 1) * 128],
                    rhs=ct_sb[:, t, nh * 512 : (nh + 1) * 512],
                    start=(t == 0), stop=(t == 3),
                )
        ost = osp.tile([128, T], F32)
        nc.scalar.copy(out=ost[:], in_=ps[:])
        nc.sync.dma_start(out=o_out[j * 128 : (j + 1) * 128, :], in_=ost[:])

    for m in range(NM):
        nc.sync.dma_start(out=a_out[m * 128 : (m + 1) * 128, :], in_=acc[:, m, :])


def build_nc():
    if "nc" in _CACHE:
        return _CACHE["nc"]
    nc = bacc.Bacc("TRN2", target_bir_lowering=False, debug=False, num_devices=8)
    xt = nc.dram_tensor("xt", [E, T], BF16, kind="ExternalInput").ap()
    wt = nc.dram_tensor("wt", [E, F], BF16, kind="ExternalInput").ap()
    wot = nc.dram_tensor("wot", [512, E], BF16, kind="ExternalInput").ap()
    bvec = nc.dram_tensor("bvec", [1, F], BF16, kind="ExternalInput").ap()
    o_out = nc.dram_tensor("o_out", [E, T], F32, kind="ExternalOutput").ap()
    a_out = nc.dram_tensor("a_out", [T, T], F32, kind="ExternalOutput").ap()
    with tile.TileContext(nc) as tc:
        with ExitStack() as ctx:
            _build_kernel(ctx, tc, xt, wt, wot, bvec, o_out, a_out)
    nc.compile()
    _CACHE["nc"] = nc
    return nc


def make_in_maps(query, W_in, b_in):
    """Host-side shard prep. Core c -> batch c//2, head-half c%2."""
    X2d = np.ascontiguousarray(np.asarray(query, dtype=np.float32).reshape(L * N, E))
    W_in = np.asarray(W_in, dtype=np.float32)
    b_in = np.asarray(b_in, dtype=np.float32)
    WT = W_in.T.copy()  # (E, 3E); Y col f <- W_in row f
    in_maps = []
    half_cache = {}
    for c in range(8):
        b, half = divmod(c, 2)
        if half not in half_cache:
            # feature order: [q rows of heads 0..7 | k rows | v rows]
            feats = np.concatenate(
                [np.arange((8 * half + hl) * 192 + off,
                           (8 * half + hl) * 192 + off + 64)
                 for off in (0, 64, 128) for hl in range(HL)]
            )
            wt = WT[:, feats].copy()
            bv = b_in[feats].copy()
            wt[:, :512] *= SCALE
            bv[:512] *= SCALE
            half_cache[half] = (
                wt.astype(NPBF16),
                bv.reshape(1, F).astype(NPBF16),
            )
        wt_bf, bv_bf = half_cache[half]
        xt = np.ascontiguousarray(X2d[b * T : (b + 1) * T].T).astype(NPBF16)
        in_maps.append({"xt": xt, "wt": wt_bf, "bvec": bv_bf})
    return in_maps


def add_wout(in_maps, W_out):
    W_out = np.asarray(W_out, dtype=np.float32)
    WoT = W_out.T.copy()  # (E, E); rows = ctx features
    for c, m in enumerate(in_maps):
        half = c % 2
        m["wot"] = np.ascontiguousarray(
            WoT[512 * half : 512 * (half + 1), :]
        ).astype(NPBF16)
    return in_maps


def kernel(query, key, value, W_in, b_in, W_out, b_out):
    nc = build_nc()
    in_maps = add_wout(make_in_maps(query, W_in, b_in), W_out)
    res = run_bass_kernel_spmd(nc, in_maps, list(range(8)))
    _CACHE["last_result"] = res
    r = res.results
    b_out = np.asarray(b_out, dtype=np.float32)
    out1 = np.empty((L, N, E), np.float32)
    out2 = np.empty((N, L, L), np.float32)
    for b in range(N):
        o = (r[2 * b]["o_out"] + r[2 * b + 1]["o_out"]).T + b_out
        out1[:, b, :] = o
        out2[b] = r[2 * b]["a_out"] + r[2 * b + 1]["a_out"]
    return out1, out2


# revision 25
# speedup vs baseline: 5.2170x; 5.2170x over previous
"""Multi-head attention (torch-bug-faithful) Bass/Tile kernel for 8 trn2 cores.

Reference math (after the torch .reshape reinterpretation):
  X2d = query.reshape(4096, 1024)                    # rows r = l*4 + n
  Y   = X2d @ W_in.T + b_in                          # (4096, 3072)
  For "batch" b in 0..4: Yb = Y[b*1024:(b+1)*1024]   # (1024, 3072)
    head h in 0..16: q = Yb[:, h*192:h*192+64] * hd**-0.5
                     k = Yb[:, h*192+64:h*192+128]
                     v = Yb[:, h*192+128:h*192+192]
    S = q @ k.T; A = softmax(S, -1); ctx_h = A @ v
  C_b[:, h*64:(h+1)*64] = ctx_h; O_b = C_b @ W_out.T + b_out
  out1[l', n', :] = O_{n'}[l']                       # (1024, 4, 1024)
  out2[b] = sum_h A_bh / 16                          # (4, 1024, 1024)

Sharding: core c -> (b = c//2, half = c%2) handles 8 heads of one batch.
Each core computes partial O.T (features x tokens) and partial out2; the
host sums the two half-partials per batch and adds b_out.

On-chip layout: all activations transposed (features on partitions).
  in_proj: Y.T tiles = W.T-chunk.T @ X.T-chunk (PE), bias via ones-row matmul
  QK:      S (q-tile x s) = (Q.T chunk).T @ K.T      (contract d=64)
  exp:     ACT Exp PSUM->SBUF bf16 with accum_out row sums
  out2:    acc += E * recip16 (scalar_tensor_tensor, DVE + GPSIMD split)
  E.T:     DMA xbar transpose (bf16) for the A@V contraction
  AV:      ctx(q-tile, d) = (E.T chunk).T @ V-chunk  (PSUM accumulated)
  norm:    ACT Copy with per-partition scale = 1/rowsum
  out_proj: O.T = (W_out.T chunk).T @ C.T            (C.T via DMA transpose)
"""

import numpy as np
import ml_dtypes
from contextlib import ExitStack

import concourse.bass as bass
import concourse.tile as tile
import concourse.mybir as mybir
from concourse import bacc
from concourse.bass_utils import run_bass_kernel_spmd

DT16 = mybir.dt.float16
F32 = mybir.dt.float32
NP16 = np.float16

L, N, E = 1024, 4, 1024
HEADS_TOTAL, HD = 16, 64
T = 1024          # tokens per core
HL = 8            # heads per core
F = HL * 3 * HD   # 1536 in_proj output features per core
KC = E // 128     # 8 contraction chunks
NM = T // 128     # 8 q tiles
POOL_M0 = 6       # out2 m-slices >= this go to GPSIMD
SCALE = float(HD) ** -0.5

_CACHE = {}


def make_pools(ctx: ExitStack, tc):
    return (
        ctx.enter_context(tc.tile_pool(name="consts", bufs=1)),
        ctx.enter_context(tc.tile_pool(name="wacc", bufs=1)),
        ctx.enter_context(tc.tile_pool(name="xet", bufs=3)),
        ctx.enter_context(tc.tile_pool(name="ytp", bufs=1)),
        ctx.enter_context(tc.tile_pool(name="ep", bufs=2)),
        ctx.enter_context(tc.tile_pool(name="vp", bufs=2)),
        ctx.enter_context(tc.tile_pool(name="cp", bufs=1)),
        ctx.enter_context(tc.tile_pool(name="ctp", bufs=1)),
        ctx.enter_context(tc.tile_pool(name="osp", bufs=2)),
        ctx.enter_context(tc.tile_pool(name="rsp", bufs=4)),
        ctx.enter_context(tc.tile_pool(name="ptmp", bufs=2)),
        ctx.enter_context(tc.tile_pool(name="accp", bufs=1)),
        ctx.enter_context(tc.tile_pool(name="mmp", bufs=3, space="PSUM")),
        ctx.enter_context(tc.tile_pool(name="ctxp", bufs=2, space="PSUM")),
    )


def _build_kernel(ctx: ExitStack, tc, xt, wt, wot, bvec, o_out, a_out,
                  pools=None):
    nc = tc.nc
    mult, add = mybir.AluOpType.mult, mybir.AluOpType.add
    Exp = mybir.ActivationFunctionType.Exp
    Ident = mybir.ActivationFunctionType.Identity

    if pools is None:
        pools = make_pools(ctx, tc)
    (consts, wacc, xet, ytp, ep, vp, cp, ctp, osp, rsp, pool_tmp, accp,
     mmp, ctxp) = pools

    # ---- input loads -----------------------------------------------------
    wt_sb = wacc.tile([128, KC, F], DT16, tag="wacc")
    xt_sb = xet.tile([128, KC, T], DT16, tag="xet")
    for c in range(KC):  # interleave so in_proj's chunk-c matmuls start early
        nc.sync.dma_start(out=wt_sb[:, c, :], in_=wt[c * 128 : (c + 1) * 128, :])
        nc.sync.dma_start(out=xt_sb[:, c, :], in_=xt[c * 128 : (c + 1) * 128, :])
    wot_sb = consts.tile([128, 4, E], DT16)
    for c in range(4):
        nc.sync.dma_start(out=wot_sb[:, c, :], in_=wot[c * 128 : (c + 1) * 128, :])
    bias_sb = consts.tile([128, F // 128], F32)
    nc.sync.dma_start(out=bias_sb[:], in_=bvec[:])

    # ---- in_proj + attention, interleaved ---------------------------------
    # f-tile group g = (g, 4+g, 8+g) holds q/k/v rows for heads 2g, 2g+1.
    # Emitting group g, then those heads' QK/exp, then the PREVIOUS pair's
    # A@V keeps the PE stream stall-free while ACT exps and DMA transposes
    # for the current pair run behind it.
    yt_sb = ytp.tile([128, F // 128, T], DT16)
    acc = accp.tile([128, NM, T], DT16)

    def seg(row):  # 64-row feature segment -> (partition0, ftile)
        return row % 128, row // 128

    def emit_ftile(j):
        ps = mmp.tile([128, T], F32, tag="mm")
        for nh in range(2):
            half = ps[:, nh * 512 : (nh + 1) * 512]
            for c in range(KC):
                nc.tensor.matmul(
                    half,
                    lhsT=wt_sb[:, c, j * 128 : (j + 1) * 128],
                    rhs=xt_sb[:, c, nh * 512 : (nh + 1) * 512],
                    start=(c == 0), stop=(c == KC - 1),
                )
        # bias folded into the PSUM->SBUF cast (per-partition bias)
        if j < 8:
            nc.scalar.activation(out=yt_sb[:, j, :], in_=ps[:], func=Ident,
                                 bias=bias_sb[:, j : j + 1])
        else:  # v tiles: DVE has slack, ACT is the scarcer engine
            nc.vector.tensor_scalar_add(yt_sb[:, j, :], ps[:],
                                        bias_sb[:, j : j + 1])

    head_state = {}

    def emit_head_front(h):
        pq, fq = seg(h * 64)
        pk, fk = seg(512 + h * 64)
        pv, fv = seg(1024 + h * 64)
        e_h = ep.tile([128, NM, T], DT16)
        rs_h = rsp.tile([128, NM], F32, tag="rs")
        for m in range(NM):
            ps = mmp.tile([128, T], F32, tag="mm")
            for nh in range(2):
                nc.tensor.matmul(
                    ps[:, nh * 512 : (nh + 1) * 512],
                    lhsT=yt_sb[pq : pq + 64, fq, m * 128 : (m + 1) * 128],
                    rhs=yt_sb[pk : pk + 64, fk, nh * 512 : (nh + 1) * 512],
                    start=True, stop=True,
                )
            nc.scalar.activation(
                out=e_h[:, m, :], in_=ps[:], func=Exp,
                accum_out=rs_h[:, m : m + 1],
            )
        recip_h = rsp.tile([128, NM], F32, tag="recip")
        recip16_h = rsp.tile([128, NM], F32, tag="recip16")
        nc.vector.reciprocal(out=recip_h[:], in_=rs_h[:])
        nc.vector.tensor_scalar_mul(recip16_h[:], recip_h[:], 1.0 / 16.0)

        # out2 partial: acc[:, m, :] (+)= E * (1/(16*rowsum)).
        # m < POOL_M0 on DVE (fused sTT); rest on otherwise-idle GPSIMD.
        for m in range(NM):
            if m < POOL_M0:
                if h == 0:
                    nc.vector.tensor_scalar_mul(
                        acc[:, m, :], e_h[:, m, :], recip16_h[:, m : m + 1]
                    )
                else:
                    nc.vector.scalar_tensor_tensor(
                        out=acc[:, m, :], in0=e_h[:, m, :],
                        scalar=recip16_h[:, m : m + 1], in1=acc[:, m, :],
                        op0=mult, op1=add,
                    )
            else:
                if h == 0:
                    nc.gpsimd.tensor_scalar_mul(
                        acc[:, m, :], e_h[:, m, :], recip16_h[:, m : m + 1]
                    )
                else:
                    tmp = pool_tmp.tile([128, T], DT16, tag="ptmp")
                    nc.gpsimd.tensor_scalar_mul(
                        tmp[:], e_h[:, m, :], recip16_h[:, m : m + 1]
                    )
                    nc.gpsimd.tensor_tensor(
                        out=acc[:, m, :], in0=acc[:, m, :], in1=tmp[:],
                        op=add,
                    )

        # V (s on partitions) and E.T via one batched xbar transpose each.
        # et_h[p, m*8+t, c] = E[m*128+c, t*128+p] = E.T chunk (t, m).
        v_h = vp.tile([128, NM, HD], DT16)
        nc.sync.dma_start_transpose(out=v_h[:], in_=yt_sb[pv : pv + 64, fv, :])
        et_h = xet.tile([128, NM * NM, 128], DT16, tag="xet")
        nc.sync.dma_start_transpose(out=et_h[:], in_=e_h[:])
        head_state[h] = (et_h, v_h, recip_h)

    c_sb = cp.tile([128, NM, 512], DT16)

    def emit_head_back(h):
        et_h, v_h, recip_h = head_state.pop(h)
        cx = ctxp.tile([128, NM, HD], F32)
        for m in range(NM):
            for t in range(NM):
                nc.tensor.matmul(
                    cx[:, m, :],
                    lhsT=et_h[:, m * NM + t, :],
                    rhs=v_h[:, t, :],
                    start=(t == 0), stop=(t == NM - 1),
                )
        # normalize rows and pack C (q on partitions, 512 ctx features)
        for m in range(NM):
            nc.vector.tensor_scalar_mul(
                c_sb[:, m, h * HD : (h + 1) * HD], cx[:, m, :],
                recip_h[:, m : m + 1],
            )

    for g in range(4):
        for j in (g, 4 + g, 8 + g):
            emit_ftile(j)
        emit_head_front(2 * g)
        emit_head_front(2 * g + 1)
        if g > 0:
            emit_head_back(2 * g - 2)
            emit_head_back(2 * g - 1)
    emit_head_back(6)
    emit_head_back(7)

    # ---- out_proj --------------------------------------------------------
    # one batched transpose: ct2[p, m*4+t, c'] = C[m*128+c', t*128+p]
    ct_sb = ctp.tile([128, 4 * NM, 128], DT16)
    nc.sync.dma_start_transpose(out=ct_sb[:], in_=c_sb[:])
    ct_r = ct_sb.rearrange("p (m t) c -> p t m c", t=4)
    for j in range(E // 128):
        ps = mmp.tile([128, T], F32, tag="mm")
        for nh in range(2):
            for t in range(4):
                nc.tensor.matmul(
                    ps[:, nh * 512 : (nh + 1) * 512],
                    lhsT=wot_sb[:, t, j * 128 : (j + 1) * 128],
                    rhs=ct_r[:, t, nh * 4 : (nh + 1) * 4, :],
                    start=(t == 0), stop=(t == 3),
                )
        ost = osp.tile([128, T], DT16)
        nc.vector.tensor_copy(ost[:], ps[:])
        nc.sync.dma_start(out=o_out[j * 128 : (j + 1) * 128, :], in_=ost[:])

    for m in range(NM):
        nc.sync.dma_start(out=a_out[m * 128 : (m + 1) * 128, :], in_=acc[:, m, :])


def build_nc(reps=1):
    key = ("nc", reps)
    if key in _CACHE:
        return _CACHE[key]
    nc = bacc.Bacc("TRN2", target_bir_lowering=False, debug=False, num_devices=8)
    xt = nc.dram_tensor("xt", [E, T], DT16, kind="ExternalInput").ap()
    wt = nc.dram_tensor("wt", [E, F], DT16, kind="ExternalInput").ap()
    wot = nc.dram_tensor("wot", [512, E], DT16, kind="ExternalInput").ap()
    bvec = nc.dram_tensor("bvec", [128, F // 128], F32, kind="ExternalInput").ap()
    o_out = nc.dram_tensor("o_out", [E, T], DT16, kind="ExternalOutput").ap()
    a_out = nc.dram_tensor("a_out", [T, T], DT16, kind="ExternalOutput").ap()
    with tile.TileContext(nc) as tc:
        with ExitStack() as ctx:
            pools = make_pools(ctx, tc)
            for _ in range(reps):
                _build_kernel(ctx, tc, xt, wt, wot, bvec, o_out, a_out,
                              pools=pools)
    nc.compile()
    _CACHE[key] = nc
    return nc


def make_in_maps(query, W_in, b_in):
    """Host-side shard prep. Core c -> batch c//2, head-half c%2."""
    X2d = np.ascontiguousarray(np.asarray(query, dtype=np.float32).reshape(L * N, E))
    W_in = np.asarray(W_in, dtype=np.float32)
    b_in = np.asarray(b_in, dtype=np.float32)
    WT = W_in.T.copy()  # (E, 3E); Y col f <- W_in row f
    in_maps = []
    half_cache = {}
    for c in range(8):
        b, half = divmod(c, 2)
        if half not in half_cache:
            # feature order: [q rows of heads 0..7 | k rows | v rows]
            feats = np.concatenate(
                [np.arange((8 * half + hl) * 192 + off,
                           (8 * half + hl) * 192 + off + 64)
                 for off in (0, 64, 128) for hl in range(HL)]
            )
            wt = WT[:, feats].copy()
            bv = b_in[feats].copy()
            wt[:, :512] *= SCALE
            bv[:512] *= SCALE
            half_cache[half] = (
                wt.astype(NP16),
                np.ascontiguousarray(bv.reshape(F // 128, 128).T
                                     ).astype(np.float32),
            )
        wt_bf, bv_bf = half_cache[half]
        xt = np.ascontiguousarray(X2d[b * T : (b + 1) * T].T).astype(NP16)
        in_maps.append({"xt": xt, "wt": wt_bf, "bvec": bv_bf})
    return in_maps


def add_wout(in_maps, W_out):
    W_out = np.asarray(W_out, dtype=np.float32)
    WoT = W_out.T.copy()  # (E, E); rows = ctx features
    for c, m in enumerate(in_maps):
        half = c % 2
        m["wot"] = np.ascontiguousarray(
            WoT[512 * half : 512 * (half + 1), :]
        ).astype(NP16)
    return in_maps


def kernel(query, key, value, W_in, b_in, W_out, b_out):
    nc = build_nc()
    in_maps = add_wout(make_in_maps(query, W_in, b_in), W_out)
    res = run_bass_kernel_spmd(nc, in_maps, list(range(8)))
    _CACHE["last_result"] = res
    r = res.results
    b_out = np.asarray(b_out, dtype=np.float32)
    out1 = np.empty((L, N, E), np.float32)
    out2 = np.empty((N, L, L), np.float32)
    for b in range(N):
        o = (r[2 * b]["o_out"].astype(np.float32)
             + r[2 * b + 1]["o_out"].astype(np.float32)).T + b_out
        out1[:, b, :] = o
        out2[b] = (r[2 * b]["a_out"].astype(np.float32)
                   + r[2 * b + 1]["a_out"].astype(np.float32))
    return out1, out2


# revision 30
# speedup vs baseline: 5.7135x; 1.0952x over previous
"""Multi-head attention (torch-bug-faithful) Bass/Tile kernel for 8 trn2 cores.

Reference math (after the torch .reshape reinterpretation):
  X2d = query.reshape(4096, 1024)                    # rows r = l*4 + n
  Y   = X2d @ W_in.T + b_in                          # (4096, 3072)
  For "batch" b in 0..4: Yb = Y[b*1024:(b+1)*1024]   # (1024, 3072)
    head h in 0..16: q = Yb[:, h*192:h*192+64] * hd**-0.5
                     k = Yb[:, h*192+64:h*192+128]
                     v = Yb[:, h*192+128:h*192+192]
    S = q @ k.T; A = softmax(S, -1); ctx_h = A @ v
  C_b[:, h*64:(h+1)*64] = ctx_h; O_b = C_b @ W_out.T + b_out
  out1[l', n', :] = O_{n'}[l']                       # (1024, 4, 1024)
  out2[b] = sum_h A_bh / 16                          # (4, 1024, 1024)

Sharding: core c -> (b = c//2, half = c%2) handles 8 heads of one batch.
Each core computes partial O.T (features x tokens) and partial out2; the
host sums the two half-partials per batch and adds b_out.

On-chip layout: all activations transposed (features on partitions).
  in_proj: Y.T tiles = W.T-chunk.T @ X.T-chunk (PE), bias via ones-row matmul
  QK:      S (q-tile x s) = (Q.T chunk).T @ K.T      (contract d=64)
  exp:     ACT Exp PSUM->SBUF bf16 with accum_out row sums
  out2:    acc += E * recip16 (scalar_tensor_tensor, DVE + GPSIMD split)
  E.T:     DMA xbar transpose (bf16) for the A@V contraction
  AV:      ctx(q-tile, d) = (E.T chunk).T @ V-chunk  (PSUM accumulated)
  norm:    ACT Copy with per-partition scale = 1/rowsum
  out_proj: O.T = (W_out.T chunk).T @ C.T            (C.T via DMA transpose)
"""

import numpy as np
import ml_dtypes
from contextlib import ExitStack

import concourse.bass as bass
import concourse.tile as tile
import concourse.mybir as mybir
from concourse import bacc
from concourse.bass_utils import run_bass_kernel_spmd

DT16 = mybir.dt.float16
F32 = mybir.dt.float32
NP16 = np.float16

L, N, E = 1024, 4, 1024
HEADS_TOTAL, HD = 16, 64
T = 1024          # tokens per core
HL = 8            # heads per core
F = HL * 3 * HD   # 1536 in_proj output features per core
KC = E // 128     # 8 contraction chunks
NM = T // 128     # 8 q tiles
POOL_M0 = 6       # out2 m-slices >= this go to GPSIMD
SCALE = float(HD) ** -0.5

_CACHE = {}


def make_pools(ctx: ExitStack, tc):
    return (
        ctx.enter_context(tc.tile_pool(name="consts", bufs=1)),
        ctx.enter_context(tc.tile_pool(name="wtp", bufs=1)),
        ctx.enter_context(tc.tile_pool(name="xtp", bufs=1)),
        ctx.enter_context(tc.tile_pool(name="etp", bufs=4)),
        ctx.enter_context(tc.tile_pool(name="ytp", bufs=1)),
        ctx.enter_context(tc.tile_pool(name="ep", bufs=4)),
        ctx.enter_context(tc.tile_pool(name="vp", bufs=2)),
        ctx.enter_context(tc.tile_pool(name="cp", bufs=1)),
        ctx.enter_context(tc.tile_pool(name="ctp", bufs=2)),
        ctx.enter_context(tc.tile_pool(name="osp", bufs=2)),
        ctx.enter_context(tc.tile_pool(name="rsp", bufs=4)),
        ctx.enter_context(tc.tile_pool(name="ptmp", bufs=2)),
        ctx.enter_context(tc.tile_pool(name="accp", bufs=1)),
        ctx.enter_context(tc.tile_pool(name="o1p", bufs=1)),
        ctx.enter_context(tc.tile_pool(name="mmp", bufs=3, space="PSUM")),
        ctx.enter_context(tc.tile_pool(name="ctxp", bufs=2, space="PSUM")),
    )


def _build_kernel(ctx: ExitStack, tc, xt, wt, wot, bvec, o_out, a_out,
                  pools=None):
    nc = tc.nc
    mult, add = mybir.AluOpType.mult, mybir.AluOpType.add
    Exp = mybir.ActivationFunctionType.Exp
    Ident = mybir.ActivationFunctionType.Identity

    if pools is None:
        pools = make_pools(ctx, tc)
    (consts, wtp, xtp, etp, ytp, ep, vp, cp, ctp, osp, rsp, pool_tmp, accp,
     o1p, mmp, ctxp) = pools

    # ---- input loads -----------------------------------------------------
    wt_sb = wtp.tile([128, KC, F], DT16)
    xt_sb = xtp.tile([128, KC, T], DT16)
    for c in range(KC):  # interleave so in_proj's chunk-c matmuls start early
        nc.sync.dma_start(out=wt_sb[:, c, :], in_=wt[c * 128 : (c + 1) * 128, :])
        nc.sync.dma_start(out=xt_sb[:, c, :], in_=xt[c * 128 : (c + 1) * 128, :])
    wot_sb = consts.tile([128, 4, E], DT16)
    for c in range(4):
        nc.sync.dma_start(out=wot_sb[:, c, :], in_=wot[c * 128 : (c + 1) * 128, :])
    bias_sb = consts.tile([128, F // 128], F32)
    nc.sync.dma_start(out=bias_sb[:], in_=bvec[:])

    # ---- in_proj + attention, interleaved --------------------------------
    # f-tile group g = (g, 4+g, 8+g) holds q/k/v rows for heads 2g, 2g+1.
    # Emitting group g, then those heads' QK/exp, then the PREVIOUS pair's
    # A@V keeps the PE stream stall-free while ACT exps and DMA transposes
    # for the current pair run behind it.
    yt_sb = ytp.tile([128, F // 128, T], DT16)
    acc = accp.tile([128, NM, T], DT16)

    def seg(row):  # 64-row feature segment -> (partition0, ftile)
        return row % 128, row // 128

    def emit_ftile(j):
        ps = mmp.tile([128, T], F32, tag="mm")
        for nh in range(2):
            half = ps[:, nh * 512 : (nh + 1) * 512]
            for c in range(KC):
                nc.tensor.matmul(
                    half,
                    lhsT=wt_sb[:, c, j * 128 : (j + 1) * 128],
                    rhs=xt_sb[:, c, nh * 512 : (nh + 1) * 512],
                    start=(c == 0), stop=(c == KC - 1),
                )
        # bias folded into the PSUM->SBUF cast (per-partition bias)
        if j < 8:
            nc.scalar.activation(out=yt_sb[:, j, :], in_=ps[:], func=Ident,
                                 bias=bias_sb[:, j : j + 1])
        else:  # v tiles: DVE has slack, ACT is the scarcer engine
            nc.vector.tensor_scalar_add(yt_sb[:, j, :], ps[:],
                                        bias_sb[:, j : j + 1])

    head_state = {}

    def emit_head_front(h):
        pq, fq = seg(h * 64)
        pk, fk = seg(512 + h * 64)
        pv, fv = seg(1024 + h * 64)
        # E split into m-halves for finer buffer recycling (ACT can run
        # ahead while older halves are still being consumed).
        e_half = [ep.tile([128, NM // 2, T], DT16, tag="e", name=f"e_{h}_{i}")
                  for i in range(2)]
        et_half = [etp.tile([128, NM * NM // 2, 128], DT16, tag="et",
                            name=f"et_{h}_{i}") for i in range(2)]
        rs_h = rsp.tile([128, NM], F32, tag="rs")
        for m in range(NM):
            hh, ml = divmod(m, NM // 2)
            ps = mmp.tile([128, T], F32, tag="mm")
            for nh in range(2):
                nc.tensor.matmul(
                    ps[:, nh * 512 : (nh + 1) * 512],
                    lhsT=yt_sb[pq : pq + 64, fq, m * 128 : (m + 1) * 128],
                    rhs=yt_sb[pk : pk + 64, fk, nh * 512 : (nh + 1) * 512],
                    start=True, stop=True,
                )
            nc.scalar.activation(
                out=e_half[hh][:, ml, :], in_=ps[:], func=Exp,
                accum_out=rs_h[:, m : m + 1],
            )
            if m == NM // 2 - 1 or m == NM - 1:
                # et[p, ml*8+t, c] = E[(hh*4+ml)*128+c, t*128+p]
                nc.sync.dma_start_transpose(out=et_half[hh][:],
                                            in_=e_half[hh][:])
        recip_h = rsp.tile([128, NM], F32, tag="recip")
        recip16_h = rsp.tile([128, NM], F32, tag="recip16")
        nc.vector.reciprocal(out=recip_h[:], in_=rs_h[:])
        nc.vector.tensor_scalar_mul(recip16_h[:], recip_h[:], 1.0 / 16.0)

        # out2 partial: acc[:, m, :] (+)= E * (1/(16*rowsum)).
        # m < POOL_M0 on DVE (fused sTT); rest on otherwise-idle GPSIMD.
        for m in range(NM):
            hh, ml = divmod(m, NM // 2)
            e_m = e_half[hh][:, ml, :]
            if m < POOL_M0:
                if h == 0:
                    nc.vector.tensor_scalar_mul(
                        acc[:, m, :], e_m, recip16_h[:, m : m + 1]
                    )
                else:
                    nc.vector.scalar_tensor_tensor(
                        out=acc[:, m, :], in0=e_m,
                        scalar=recip16_h[:, m : m + 1], in1=acc[:, m, :],
                        op0=mult, op1=add,
                    )
            else:
                if h == 0:
                    nc.gpsimd.tensor_scalar_mul(
                        acc[:, m, :], e_m, recip16_h[:, m : m + 1]
                    )
                else:
                    tmp = pool_tmp.tile([128, T], DT16, tag="ptmp")
                    nc.gpsimd.tensor_scalar_mul(
                        tmp[:], e_m, recip16_h[:, m : m + 1]
                    )
                    nc.gpsimd.tensor_tensor(
                        out=acc[:, m, :], in0=acc[:, m, :], in1=tmp[:],
                        op=add,
                    )

        v_h = vp.tile([128, NM, HD], DT16)
        nc.sync.dma_start_transpose(out=v_h[:], in_=yt_sb[pv : pv + 64, fv, :])
        head_state[h] = (et_half, v_h, recip_h)

    c_sb = cp.tile([128, NM, 512], DT16)

    def emit_head_back(h):
        et_half, v_h, recip_h = head_state.pop(h)
        cx = ctxp.tile([128, NM, HD], F32)
        for m in range(NM):
            hh, ml = divmod(m, NM // 2)
            for t in range(NM):
                nc.tensor.matmul(
                    cx[:, m, :],
                    lhsT=et_half[hh][:, ml * NM + t, :],
                    rhs=v_h[:, t, :],
                    start=(t == 0), stop=(t == NM - 1),
                )
        # normalize rows and pack C (q on partitions, 512 ctx features)
        for m in range(NM):
            nc.vector.tensor_scalar_mul(
                c_sb[:, m, h * HD : (h + 1) * HD], cx[:, m, :],
                recip_h[:, m : m + 1],
            )

    o1_sb = o1p.tile([128, E // 128, T], DT16)

    def emit_outproj_pass(p):
        # pass p covers W_out.T chunks t = 2p, 2p+1 <-> heads 4p .. 4p+3.
        # ct[pp, tt, m*128+c'] = C[m*128+c', (2p+tt)*128+pp]
        ct_sb = ctp.tile([128, 2, T], DT16, tag="ct")
        for m in range(NM):
            nc.sync.dma_start_transpose(
                out=ct_sb[:, :, m * 128 : (m + 1) * 128],
                in_=c_sb[:, m, p * 256 : (p + 1) * 256],
            )
        for j in range(E // 128):
            ps = mmp.tile([128, T], F32, tag="mm")
            for nh in range(2):
                for tt in range(2):
                    nc.tensor.matmul(
                        ps[:, nh * 512 : (nh + 1) * 512],
                        lhsT=wot_sb[:, 2 * p + tt, j * 128 : (j + 1) * 128],
                        rhs=ct_sb[:, tt, nh * 512 : (nh + 1) * 512],
                        start=(tt == 0), stop=(tt == 1),
                    )
            if p == 0:
                nc.vector.tensor_copy(o1_sb[:, j, :], ps[:])
            else:
                ost = osp.tile([128, T], DT16)
                nc.vector.tensor_tensor(out=ost[:], in0=ps[:],
                                        in1=o1_sb[:, j, :], op=add)
                nc.sync.dma_start(out=o_out[j * 128 : (j + 1) * 128, :],
                                  in_=ost[:])

    for g in range(4):
        for j in (g, 4 + g, 8 + g):
            emit_ftile(j)
        emit_head_front(2 * g)
        emit_head_front(2 * g + 1)
        if g > 0:
            emit_head_back(2 * g - 2)
            emit_head_back(2 * g - 1)
    emit_outproj_pass(0)  # heads 0-3 ready; PE is otherwise draining here
    emit_head_back(6)
    emit_head_back(7)
    emit_outproj_pass(1)

    for m in range(NM):
        nc.sync.dma_start(out=a_out[m * 128 : (m + 1) * 128, :], in_=acc[:, m, :])


def build_nc(reps=1):
    key = ("nc", reps)
    if key in _CACHE:
        return _CACHE[key]
    nc = bacc.Bacc("TRN2", target_bir_lowering=False, debug=False, num_devices=8)
    xt = nc.dram_tensor("xt", [E, T], DT16, kind="ExternalInput").ap()
    wt = nc.dram_tensor("wt", [E, F], DT16, kind="ExternalInput").ap()
    wot = nc.dram_tensor("wot", [512, E], DT16, kind="ExternalInput").ap()
    bvec = nc.dram_tensor("bvec", [128, F // 128], F32, kind="ExternalInput").ap()
    o_out = nc.dram_tensor("o_out", [E, T], DT16, kind="ExternalOutput").ap()
    a_out = nc.dram_tensor("a_out", [T, T], DT16, kind="ExternalOutput").ap()
    with tile.TileContext(nc) as tc:
        with ExitStack() as ctx:
            pools = make_pools(ctx, tc)
            for _ in range(reps):
                _build_kernel(ctx, tc, xt, wt, wot, bvec, o_out, a_out,
                              pools=pools)
    nc.compile()
    _CACHE[key] = nc
    return nc


def make_in_maps(query, W_in, b_in):
    """Host-side shard prep. Core c -> batch c//2, head-half c%2."""
    X2d = np.ascontiguousarray(np.asarray(query, dtype=np.float32).reshape(L * N, E))
    W_in = np.asarray(W_in, dtype=np.float32)
    b_in = np.asarray(b_in, dtype=np.float32)
    WT = W_in.T.copy()  # (E, 3E); Y col f <- W_in row f
    in_maps = []
    half_cache = {}
    for c in range(8):
        b, half = divmod(c, 2)
        if half not in half_cache:
            # feature order: [q rows of heads 0..7 | k rows | v rows]
            feats = np.concatenate(
                [np.arange((8 * half + hl) * 192 + off,
                           (8 * half + hl) * 192 + off + 64)
                 for off in (0, 64, 128) for hl in range(HL)]
            )
            wt = WT[:, feats].copy()
            bv = b_in[feats].copy()
            wt[:, :512] *= SCALE
            bv[:512] *= SCALE
            half_cache[half] = (
                wt.astype(NP16),
                np.ascontiguousarray(bv.reshape(F // 128, 128).T
                                     ).astype(np.float32),
            )
        wt_bf, bv_bf = half_cache[half]
        xt = np.ascontiguousarray(X2d[b * T : (b + 1) * T].T).astype(NP16)
        in_maps.append({"xt": xt, "wt": wt_bf, "bvec": bv_bf})
    return in_maps


def add_wout(in_maps, W_out):
    W_out = np.asarray(W_out, dtype=np.float32)
    WoT = W_out.T.copy()  # (E, E); rows = ctx features
    for c, m in enumerate(in_maps):
        half = c % 2
        m["wot"] = np.ascontiguousarray(
            WoT[512 * half : 512 * (half + 1), :]
        ).astype(NP16)
    return in_maps


def kernel(query, key, value, W_in, b_in, W_out, b_out):
    nc = build_nc()
    in_maps = add_wout(make_in_maps(query, W_in, b_in), W_out)
    res = run_bass_kernel_spmd(nc, in_maps, list(range(8)))
    _CACHE["last_result"] = res
    r = res.results
    b_out = np.asarray(b_out, dtype=np.float32)
    out1 = np.empty((L, N, E), np.float32)
    out2 = np.empty((N, L, L), np.float32)
    for b in range(N):
        o = (r[2 * b]["o_out"].astype(np.float32)
             + r[2 * b + 1]["o_out"].astype(np.float32)).T + b_out
        out1[:, b, :] = o
        out2[b] = (r[2 * b]["a_out"].astype(np.float32)
                   + r[2 * b + 1]["a_out"].astype(np.float32))
    return out1, out2
